# revision 10
# baseline (speedup 1.0000x reference)
"""Bass/Trainium2 SPMD kernel for a 2-layer GCN encoder.

Math (per reference):
    src/dst = edges + self-loops
    deg[v]  = #edges with dst==v (incl self-loop);  dinv = 1/sqrt(deg)
    layer(x, W, b): out[d] = dinv[d] * sum_{e: dst_e==d} dinv[src_e] * (x@W)[src_e] + b
    y = layer1(sigmoid(layer0(x, W0, b0)), W1, b1)

Distribution: nodes are sharded contiguously across 8 cores (6250 each).
Edges are owned by the destination core.  Each core:
  1. GEMM on its x rows, pre-scales rows by dinv (so the per-edge weight
     dinv[src]*dinv[dst] factorizes into a row pre-scale and an output
     post-scale), AllGathers the scaled features.
  2. For each 128-row destination block, gathers the source rows of its
     edges (dma_gather, int16 indices => the node table is split in two
     halves), builds one-hot scatter matrices on the vector engine
     (iota == slot), and scatter-adds via TensorE matmuls accumulating in
     PSUM.  Bias enters as a rank-1 matmul (sqrt(deg) x b), so the final
     PSUM->SBUF copy can apply the dinv post-scale (and sigmoid) in one
     ScalarE activation.
"""

import math

import numpy as np

import concourse.bacc as bacc
import concourse.bass as bass
import concourse.mybir as mybir
import concourse.tile as tile
from concourse.bass_utils import run_bass_kernel_spmd

P = 128
F32 = mybir.dt.float32
I16 = mybir.dt.int16

# Full-problem constants
N_NODES = 50000
N_CORES = 8
F0, F1, F2 = 128, 128, 64
GROUP_BLOCKS = 4  # dst blocks per dma_gather batch


def _round_up(x, m):
    return (x + m - 1) // m * m


class Plan:
    """Compile-time schedule, identical across cores (SPMD)."""

    def __init__(self, n_nodes, n_cores, gb):
        assert n_nodes % n_cores == 0
        self.n_nodes = n_nodes
        self.n_cores = n_cores
        self.npc = n_nodes // n_cores
        self.nblk = math.ceil(self.npc / P)
        self.hb = (n_nodes + 1) // 2  # half boundary for int16 gather indices
        assert self.hb <= 32768
        self.gb = gb
        self.groups = [
            list(range(i, min(i + gb, self.nblk))) for i in range(0, self.nblk, gb)
        ]
        # filled by finalize(): per-(blk, half) uniform padded sizes
        self.SZ = None  # [nblk, 2] int, multiples of P
        self.seg_col = {}  # (b, h) -> global chunk-column base
        self.seg_idx16 = {}  # (g_idx, h) -> int16-column base of that gather
        self.seg_ci = {}  # (b, h) -> column base within the gather dst tile
        self.gather_nid = {}  # (g_idx, h) -> num idxs
        self.ncols = 0
        self.tot16 = 0

    def finalize(self, sz):
        self.SZ = sz
        col = 0
        i16 = 0
        for gi, blocks in enumerate(self.groups):
            for h in (0, 1):
                nid = int(sum(self.SZ[b, h] for b in blocks))
                self.gather_nid[(gi, h)] = nid
                self.seg_idx16[(gi, h)] = i16
                ci = 0
                for b in blocks:
                    self.seg_col[(b, h)] = col
                    self.seg_ci[(b, h)] = ci
                    col += int(self.SZ[b, h]) // P
                    ci += int(self.SZ[b, h]) // P
                i16 += nid // 16
        self.ncols = col
        self.tot16 = i16


def _build_metadata(edges, n_nodes, n_cores, gb=GROUP_BLOCKS):
    """Host-side integer preprocessing: shard + sort edges, build gather
    indices / slot vectors / degree tables.  Returns (plan, per_core dict)."""
    plan = Plan(n_nodes, n_cores, gb)
    npc, nblk, hb = plan.npc, plan.nblk, plan.hb

    loop = np.arange(n_nodes, dtype=np.int64)
    src = np.concatenate([np.asarray(edges[0], dtype=np.int64), loop])
    dst = np.concatenate([np.asarray(edges[1], dtype=np.int64), loop])
    deg = np.bincount(dst, minlength=n_nodes).astype(np.float32)

    owner = dst // npc
    ldst = dst % npc
    blk = ldst // P
    slot = (ldst % P).astype(np.float32)
    half = (src >= hb).astype(np.int64)
    cell = ((owner * nblk) + blk) * 2 + half
    order = np.lexsort((src, cell))
    cell_s = cell[order]
    src_s = src[order]
    slot_s = slot[order]

    ncells = n_cores * nblk * 2
    counts = np.bincount(cell_s, minlength=ncells).reshape(n_cores, nblk, 2)
    starts = np.concatenate([[0], np.cumsum(counts.reshape(-1))])[:-1].reshape(
        n_cores, nblk, 2
    )
    sz = np.maximum(counts.max(axis=0), 0)
    sz = (np.ceil(sz / P).astype(np.int64)) * P  # [nblk, 2]
    plan.finalize(sz)

    tot = int(sz.sum())
    ncols = tot // P
    tot16 = tot // 16
    assert ncols == plan.ncols and tot16 == plan.tot16

    per_core = []
    for c in range(n_cores):
        idx16 = np.zeros((16, tot16), np.int16)
        slots_t = np.full((P, ncols), -1.0, np.float32)
        off = 0
        for blocks in plan.groups:
            for h in (0, 1):
                for b in blocks:
                    n = int(counts[c, b, h])
                    s0 = int(starts[c, b, h])
                    if n:
                        j = off + np.arange(n)
                        seg_src = (src_s[s0 : s0 + n] - h * hb).astype(np.int16)
                        idx16[j % 16, j // 16] = seg_src
                        slots_t[j % P, j // P] = slot_s[s0 : s0 + n]
                    off += int(sz[b, h])
        assert off == tot
        deg_loc = np.ones(nblk * P, np.float32)
        deg_loc[:npc] = deg[c * npc : (c + 1) * npc]
        deg_t = deg_loc.reshape(nblk, P).T.copy()  # [P, nblk]
        per_core.append(
            dict(
                idx16=np.tile(idx16, (8, 1)),  # [128, tot16]
                slots=slots_t,
                degt=deg_t,
                degrow=deg_loc.reshape(1, -1).copy(),
            )
        )
    return plan, per_core


def _build_nc(plan, f0, f1, f2):
    """Build the SPMD bass program (same for every core)."""
    n_nodes, npc, nblk, hb = plan.n_nodes, plan.npc, plan.nblk, plan.hb
    rows = (hb, n_nodes - hb)  # rows of each half table
    nc = bacc.Bacc(
        "TRN2", target_bir_lowering=False, debug=False, num_devices=plan.n_cores
    )

    # I/O
    xT_d = nc.dram_tensor("xT", [f0, npc], F32, kind="ExternalInput")
    w0_d = nc.dram_tensor("W0", [f0, f1], F32, kind="ExternalInput")
    w1_d = nc.dram_tensor("W1", [f1, f2], F32, kind="ExternalInput")
    b0_d = nc.dram_tensor("b0", [1, f1], F32, kind="ExternalInput")
    b1_d = nc.dram_tensor("b1", [1, f2], F32, kind="ExternalInput")
    iota_d = nc.dram_tensor("iota", [P, P], F32, kind="ExternalInput")
    ident_d = nc.dram_tensor("ident", [P, P], F32, kind="ExternalInput")
    degt_d = nc.dram_tensor("degt", [P, nblk], F32, kind="ExternalInput")
    degrow_d = nc.dram_tensor("degrow", [1, nblk * P], F32, kind="ExternalInput")
    idx_d = nc.dram_tensor("idx16", [P, plan.tot16], I16, kind="ExternalInput")
    slots_d = nc.dram_tensor("slots", [P, plan.ncols], F32, kind="ExternalInput")
    y_d = nc.dram_tensor("y", [npc, f2], F32, kind="ExternalOutput")

    rg = [list(range(plan.n_cores))]
    AF = mybir.ActivationFunctionType

    with tile.TileContext(nc) as tc:
        with (
            tc.tile_pool(name="dram", bufs=1, space="DRAM") as dramp,
            tc.tile_pool(name="const", bufs=1) as constp,
            tc.tile_pool(name="gath", bufs=4) as gpool,
            tc.tile_pool(name="sel", bufs=6) as spool,
            tc.tile_pool(name="stage", bufs=4) as stpool,
            tc.tile_pool(name="pgemm", bufs=2, space="PSUM") as pgemm,
            tc.tile_pool(name="pscat", bufs=2, space="PSUM") as pscat,
            tc.tile_pool(name="ptrans", bufs=2, space="PSUM") as ptrans,
        ):
            h1_loc = dramp.tile([npc, f1], F32, name="h1_loc")
            h1_full = dramp.tile(
                [n_nodes, f1], F32, addr_space="Shared", name="h1_full"
            )
            h2_loc = dramp.tile([npc, f2], F32, name="h2_loc")
            h2_full = dramp.tile(
                [n_nodes, f2], F32, addr_space="Shared", name="h2_full"
            )

            # ---- constants / metadata ----
            def load_const(name, dram, shape, dtype=F32):
                t = constp.tile(shape, dtype, name=name)
                nc.sync.dma_start(out=t[:], in_=dram[:])
                return t

            w0_t = load_const("w0_t", w0_d, [f0, f1])
            w1_t = load_const("w1_t", w1_d, [f1, f2])
            b0_t = load_const("b0_t", b0_d, [1, f1])
            b1_t = load_const("b1_t", b1_d, [1, f2])
            iota_t = load_const("iota_t", iota_d, [P, P])
            ident_t = load_const("ident_t", ident_d, [P, P])
            degt_t = load_const("degt_t", degt_d, [P, nblk])
            degrow_t = load_const("degrow_t", degrow_d, [1, nblk * P])
            idx_t = load_const("idx_t", idx_d, [P, plan.tot16], I16)
            slots_t = load_const("slots_t", slots_d, [P, plan.ncols])
            xT_t = load_const("xT_t", xT_d, [f0, npc])

            # dinv = 1/sqrt(deg); sqdeg rows (flat, partition 0) for bias matmuls
            sq_t = constp.tile([P, nblk], F32, name="sq_t")
            nc.scalar.activation(sq_t[:], degt_t[:], AF.Sqrt)
            dinv_t = constp.tile([P, nblk], F32, name="dinv_t")
            nc.vector.reciprocal(dinv_t[:], sq_t[:])
            sqrow_t = constp.tile([1, nblk * P], F32, name="sqrow_t")
            nc.scalar.activation(sqrow_t[:], degrow_t[:], AF.Sqrt)

            x1T_t = constp.tile([f1, nblk * P], F32, name="x1T_t")

            def gemm_layer(src_sbuf, w_t, fout, dst_dram):
                """dst_dram[rows] = dinv * (x @ W) for the local node rows."""
                for t in range(nblk):
                    wt = min(P, npc - t * P)
                    hp = pgemm.tile([P, fout], F32, name="hp")
                    nc.tensor.matmul(
                        hp[:wt, :],
                        src_sbuf[:, t * P : t * P + wt],
                        w_t[:],
                        start=True,
                        stop=True,
                    )
                    hs = stpool.tile([P, fout], F32, name="hs")
                    nc.scalar.activation(
                        hs[:wt, :],
                        hp[:wt, :],
                        AF.Copy,
                        scale=dinv_t[:wt, t : t + 1],
                    )
                    nc.sync.dma_start(
                        out=dst_dram[t * P : t * P + wt, :], in_=hs[:wt, :]
                    )

            def scatter_layer(h_full, fout, bias_t, is_last):
                """For every dst block: gather + one-hot matmul scatter-add."""
                for gi, blocks in enumerate(plan.groups):
                    gt = {}
                    for h in (0, 1):
                        nid = plan.gather_nid[(gi, h)]
                        if nid == 0:
                            continue
                        g_tile = gpool.tile(
                            [P, nid // P, fout], F32, tag="gath", name=f"g{gi}_{h}"
                        )
                        i0 = plan.seg_idx16[(gi, h)]
                        nc.gpsimd.dma_gather(
                            g_tile[:],
                            h_full[h * hb : h * hb + rows[h], :],
                            idx_t[:, i0 : i0 + nid // 16],
                            nid,
                            nid,
                            fout,
                            single_packet=False,
                        )
                        gt[h] = g_tile
                    for b in blocks:
                        wb = min(P, npc - b * P)
                        pb = pscat.tile([P, fout], F32, name="pb")
                        nc.tensor.matmul(
                            pb[:],
                            sqrow_t[0:1, b * P : (b + 1) * P],
                            bias_t[:],
                            start=True,
                            stop=False,
                        )
                        nchunks = sum(
                            int(plan.SZ[b, h]) // P for h in (0, 1)
                        )
                        k = 0
                        for h in (0, 1):
                            nch = int(plan.SZ[b, h]) // P
                            col0 = plan.seg_col.get((b, h), 0)
                            ci0 = plan.seg_ci.get((b, h), 0)
                            for ci in range(nch):
                                sel = spool.tile([P, P], F32, tag="sel", name="sel")
                                nc.vector.tensor_scalar(
                                    sel[:],
                                    iota_t[:],
                                    slots_t[:, col0 + ci : col0 + ci + 1],
                                    None,
                                    op0=mybir.AluOpType.is_equal,
                                )
                                k += 1
                                nc.tensor.matmul(
                                    pb[:],
                                    sel[:],
                                    gt[h][:, ci0 + ci, :],
                                    start=False,
                                    stop=(k == nchunks),
                                )
                        ob = stpool.tile([P, fout], F32, tag="ob", name="ob")
                        if is_last:
                            nc.scalar.activation(
                                ob[:wb, :],
                                pb[:wb, :],
                                AF.Copy,
                                scale=dinv_t[:wb, b : b + 1],
                            )
                            nc.sync.dma_start(
                                out=y_d[b * P : b * P + wb, :], in_=ob[:wb, :]
                            )
                        else:
                            nc.scalar.activation(
                                ob[:],
                                pb[:],
                                AF.Sigmoid,
                                scale=dinv_t[:, b : b + 1],
                            )
                            pt = ptrans.tile([P, P], F32, name="pt")
                            nc.tensor.transpose(pt[:], ob[:], ident_t[:])
                            nc.vector.tensor_copy(
                                x1T_t[:, b * P : (b + 1) * P], pt[:]
                            )

            # ---- layer 0 ----
            gemm_layer(xT_t, w0_t, f1, h1_loc)
            nc.gpsimd.collective_compute(
                "AllGather",
                mybir.AluOpType.bypass,
                replica_groups=rg,
                ins=[h1_loc[:, :].opt()],
                outs=[h1_full[:, :].opt()],
            )
            scatter_layer(h1_full, f1, b0_t, is_last=False)

            # ---- layer 1 ----
            gemm_layer(x1T_t, w1_t, f2, h2_loc)
            nc.gpsimd.collective_compute(
                "AllGather",
                mybir.AluOpType.bypass,
                replica_groups=rg,
                ins=[h2_loc[:, :].opt()],
                outs=[h2_full[:, :].opt()],
            )
            scatter_layer(h2_full, f2, b1_t, is_last=True)

    nc.compile()
    return nc


def _make_in_maps(x, W0, b0, W1, b1, plan, per_core):
    npc = plan.npc
    x = np.asarray(x, dtype=np.float32)
    shared = dict(
        W0=np.asarray(W0, np.float32).reshape(W0.shape[0], -1),
        W1=np.asarray(W1, np.float32).reshape(W1.shape[0], -1),
        b0=np.asarray(b0, np.float32).reshape(1, -1),
        b1=np.asarray(b1, np.float32).reshape(1, -1),
        iota=np.tile(np.arange(P, dtype=np.float32)[None, :], (P, 1)).copy(),
        ident=np.eye(P, dtype=np.float32),
    )
    in_maps = []
    for c in range(plan.n_cores):
        m = dict(shared)
        m["xT"] = np.ascontiguousarray(x[c * npc : (c + 1) * npc, :].T)
        m["idx16"] = per_core[c]["idx16"]
        m["slots"] = per_core[c]["slots"]
        m["degt"] = per_core[c]["degt"]
        m["degrow"] = per_core[c]["degrow"]
        in_maps.append(m)
    return in_maps


_CACHE = {}


def build(x, edges, W0, b0, W1, b1, n_nodes=N_NODES, n_cores=N_CORES,
          gb=GROUP_BLOCKS):
    """Returns (nc, in_maps, plan). Cached on the edge structure size."""
    plan, per_core = _build_metadata(edges, n_nodes, n_cores, gb)
    key = (n_nodes, n_cores, gb, tuple(plan.SZ.reshape(-1).tolist()))
    if key not in _CACHE:
        _CACHE[key] = _build_nc(plan, x.shape[1], W0.shape[1], W1.shape[1])
    nc = _CACHE[key]
    in_maps = _make_in_maps(x, W0, b0, W1, b1, plan, per_core)
    return nc, in_maps, plan


def kernel(x, edges, W0, b0, W1, b1):
    x = np.asarray(x)
    nc, in_maps, plan = build(x, edges, W0, b0, W1, b1)
    res = run_bass_kernel_spmd(nc, in_maps, list(range(plan.n_cores)))
    y = np.concatenate([r["y"] for r in res.results], axis=0)
    return y.astype(np.float32)


# revision 14
# speedup vs baseline: 1.1961x; 1.1961x over previous
"""Bass/Trainium2 SPMD kernel for a 2-layer GCN encoder.

Math (per reference):
    src/dst = edges + self-loops
    deg[v]  = #edges with dst==v (incl self-loop);  dinv = 1/sqrt(deg)
    layer(x, W, b): out[d] = dinv[d] * sum_{e: dst_e==d} dinv[src_e] * (x@W)[src_e] + b
    y = layer1(sigmoid(layer0(x, W0, b0)), W1, b1)

Distribution: nodes are sharded contiguously across 8 cores (6250 each).
Edges are owned by the destination core.  Each core:
  1. GEMM on its x rows, pre-scales rows by dinv (so the per-edge weight
     dinv[src]*dinv[dst] factorizes into a row pre-scale and an output
     post-scale), AllGathers the scaled features.
  2. For each 128-row destination block, gathers the source rows of its
     edges (dma_gather, int16 indices => the node table is split in two
     halves), builds one-hot scatter matrices on the vector engine
     (iota == slot), and scatter-adds via TensorE matmuls accumulating in
     PSUM.  Bias enters as a rank-1 matmul (sqrt(deg) x b), so the final
     PSUM->SBUF copy can apply the dinv post-scale (and sigmoid) in one
     ScalarE activation.
"""

import math

import numpy as np

import concourse.bacc as bacc
import concourse.bass as bass
import concourse.mybir as mybir
import concourse.tile as tile
from concourse.bass_utils import run_bass_kernel_spmd

P = 128
F32 = mybir.dt.float32
I16 = mybir.dt.int16

# Full-problem constants
N_NODES = 50000
N_CORES = 8
F0, F1, F2 = 128, 128, 64
GROUP_BLOCKS = 3  # dst blocks per dma_gather batch


def _round_up(x, m):
    return (x + m - 1) // m * m


class Plan:
    """Compile-time schedule, identical across cores (SPMD)."""

    def __init__(self, n_nodes, n_cores, gb):
        assert n_nodes % n_cores == 0
        self.n_nodes = n_nodes
        self.n_cores = n_cores
        self.npc = n_nodes // n_cores
        self.nblk = math.ceil(self.npc / P)
        self.hb = (n_nodes + 1) // 2  # half boundary for int16 gather indices
        assert self.hb <= 32768
        self.gb = gb
        self.groups = [
            list(range(i, min(i + gb, self.nblk))) for i in range(0, self.nblk, gb)
        ]
        # filled by finalize(): per-(blk, half) uniform padded sizes
        self.SZ = None  # [nblk, 2] int, multiples of P
        self.seg_col = {}  # (b, h) -> global chunk-column base
        self.seg_idx16 = {}  # (g_idx, h) -> int16-column base of that gather
        self.seg_ci = {}  # (b, h) -> column base within the gather dst tile
        self.gather_nid = {}  # (g_idx, h) -> num idxs
        self.ncols = 0
        self.tot16 = 0

    def finalize(self, sz):
        self.SZ = sz
        col = 0
        i16 = 0
        for gi, blocks in enumerate(self.groups):
            for h in (0, 1):
                nid = int(sum(self.SZ[b, h] for b in blocks))
                self.gather_nid[(gi, h)] = nid
                self.seg_idx16[(gi, h)] = i16
                ci = 0
                for b in blocks:
                    self.seg_col[(b, h)] = col
                    self.seg_ci[(b, h)] = ci
                    col += int(self.SZ[b, h]) // P
                    ci += int(self.SZ[b, h]) // P
                i16 += nid // 16
        self.ncols = col
        self.tot16 = i16


def _build_metadata(edges, n_nodes, n_cores, gb=GROUP_BLOCKS):
    """Host-side integer preprocessing: shard + sort edges, build gather
    indices / slot vectors / degree tables.  Returns (plan, per_core dict)."""
    plan = Plan(n_nodes, n_cores, gb)
    npc, nblk, hb = plan.npc, plan.nblk, plan.hb

    loop = np.arange(n_nodes, dtype=np.int64)
    src = np.concatenate([np.asarray(edges[0], dtype=np.int64), loop])
    dst = np.concatenate([np.asarray(edges[1], dtype=np.int64), loop])
    deg = np.bincount(dst, minlength=n_nodes).astype(np.float32)

    owner = dst // npc
    ldst = dst % npc
    blk = ldst // P
    slot = (ldst % P).astype(np.float32)
    half = (src >= hb).astype(np.int64)
    cell = ((owner * nblk) + blk) * 2 + half
    order = np.lexsort((src, cell))
    cell_s = cell[order]
    src_s = src[order]
    slot_s = slot[order]

    ncells = n_cores * nblk * 2
    counts = np.bincount(cell_s, minlength=ncells).reshape(n_cores, nblk, 2)
    starts = np.concatenate([[0], np.cumsum(counts.reshape(-1))])[:-1].reshape(
        n_cores, nblk, 2
    )
    sz = np.maximum(counts.max(axis=0), 0)
    sz = (np.ceil(sz / P).astype(np.int64)) * P  # [nblk, 2]
    plan.finalize(sz)

    tot = int(sz.sum())
    ncols = tot // P
    tot16 = tot // 16
    assert ncols == plan.ncols and tot16 == plan.tot16

    per_core = []
    for c in range(n_cores):
        idx16 = np.zeros((16, tot16), np.int16)
        slots_t = np.full((P, ncols), -1.0, np.float32)
        off = 0
        for blocks in plan.groups:
            for h in (0, 1):
                for b in blocks:
                    n = int(counts[c, b, h])
                    s0 = int(starts[c, b, h])
                    if n:
                        j = off + np.arange(n)
                        seg_src = (src_s[s0 : s0 + n] - h * hb).astype(np.int16)
                        idx16[j % 16, j // 16] = seg_src
                        slots_t[j % P, j // P] = slot_s[s0 : s0 + n]
                    off += int(sz[b, h])
        assert off == tot
        deg_loc = np.ones(nblk * P, np.float32)
        deg_loc[:npc] = deg[c * npc : (c + 1) * npc]
        deg_t = deg_loc.reshape(nblk, P).T.copy()  # [P, nblk]
        per_core.append(
            dict(
                idx16=np.tile(idx16, (8, 1)),  # [128, tot16]
                slots=slots_t,
                degt=deg_t,
                degrow=deg_loc.reshape(1, -1).copy(),
            )
        )
    return plan, per_core


def _build_nc(plan, f0, f1, f2):
    """Build the SPMD bass program (same for every core)."""
    n_nodes, npc, nblk, hb = plan.n_nodes, plan.npc, plan.nblk, plan.hb
    rows = (hb, n_nodes - hb)  # rows of each half table
    nc = bacc.Bacc(
        "TRN2", target_bir_lowering=False, debug=False, num_devices=plan.n_cores
    )

    # I/O
    xT_d = nc.dram_tensor("xT", [f0, npc], F32, kind="ExternalInput")
    w0_d = nc.dram_tensor("W0", [f0, f1], F32, kind="ExternalInput")
    w1_d = nc.dram_tensor("W1", [f1, f2], F32, kind="ExternalInput")
    b0_d = nc.dram_tensor("b0", [1, f1], F32, kind="ExternalInput")
    b1_d = nc.dram_tensor("b1", [1, f2], F32, kind="ExternalInput")
    iota_d = nc.dram_tensor("iota", [P, P], F32, kind="ExternalInput")
    ident_d = nc.dram_tensor("ident", [P, P], F32, kind="ExternalInput")
    degt_d = nc.dram_tensor("degt", [P, nblk], F32, kind="ExternalInput")
    degrow_d = nc.dram_tensor("degrow", [1, nblk * P], F32, kind="ExternalInput")
    idx_d = nc.dram_tensor("idx16", [P, plan.tot16], I16, kind="ExternalInput")
    slots_d = nc.dram_tensor("slots", [P, plan.ncols], F32, kind="ExternalInput")
    y_d = nc.dram_tensor("y", [npc, f2], F32, kind="ExternalOutput")

    rg = [list(range(plan.n_cores))]
    AF = mybir.ActivationFunctionType

    with tile.TileContext(nc) as tc:
        with (
            tc.tile_pool(name="dram", bufs=1, space="DRAM") as dramp,
            tc.tile_pool(name="const", bufs=1) as constp,
            tc.tile_pool(name="gath", bufs=4) as gpool,
            tc.tile_pool(name="sel", bufs=4) as spool,
            tc.tile_pool(name="stage", bufs=4) as stpool,
            tc.tile_pool(name="pgemm", bufs=2, space="PSUM") as pgemm,
            tc.tile_pool(name="pscat", bufs=2, space="PSUM") as pscat,
            tc.tile_pool(name="ptrans", bufs=2, space="PSUM") as ptrans,
        ):
            h1_loc = dramp.tile([npc, f1], F32, name="h1_loc")
            h1_full = dramp.tile(
                [n_nodes, f1], F32, addr_space="Shared", name="h1_full"
            )
            h2_loc = dramp.tile([npc, f2], F32, name="h2_loc")
            h2_full = dramp.tile(
                [n_nodes, f2], F32, addr_space="Shared", name="h2_full"
            )

            # ---- constants / metadata ----
            def load_const(name, dram, shape, dtype=F32):
                t = constp.tile(shape, dtype, name=name)
                nc.sync.dma_start(out=t[:], in_=dram[:])
                return t

            # ordered so the L0 GEMM -> AllGather chain starts ASAP; the big
            # gather metadata loads overlap with it
            xT_t = load_const("xT_t", xT_d, [f0, npc])
            w0_t = load_const("w0_t", w0_d, [f0, f1])
            degt_t = load_const("degt_t", degt_d, [P, nblk])
            w1_t = load_const("w1_t", w1_d, [f1, f2])
            b0_t = load_const("b0_t", b0_d, [1, f1])
            b1_t = load_const("b1_t", b1_d, [1, f2])
            iota_t = load_const("iota_t", iota_d, [P, P])
            ident_t = load_const("ident_t", ident_d, [P, P])
            degrow_t = load_const("degrow_t", degrow_d, [1, nblk * P])
            idx_t = load_const("idx_t", idx_d, [P, plan.tot16], I16)
            slots_t = load_const("slots_t", slots_d, [P, plan.ncols])

            # dinv = 1/sqrt(deg); sqdeg rows (flat, partition 0) for bias matmuls
            sq_t = constp.tile([P, nblk], F32, name="sq_t")
            nc.scalar.activation(sq_t[:], degt_t[:], AF.Sqrt)
            dinv_t = constp.tile([P, nblk], F32, name="dinv_t")
            nc.vector.reciprocal(dinv_t[:], sq_t[:])
            sqrow_t = constp.tile([1, nblk * P], F32, name="sqrow_t")
            nc.scalar.activation(sqrow_t[:], degrow_t[:], AF.Sqrt)

            x1T_t = constp.tile([f1, nblk * P], F32, name="x1T_t")

            def gemm_layer(src_sbuf, w_t, fout, dst_dram):
                """dst_dram[rows] = dinv * (x @ W) for the local node rows."""
                for t in range(nblk):
                    wt = min(P, npc - t * P)
                    hp = pgemm.tile([P, fout], F32, name="hp")
                    nc.tensor.matmul(
                        hp[:wt, :],
                        src_sbuf[:, t * P : t * P + wt],
                        w_t[:],
                        start=True,
                        stop=True,
                    )
                    hs = stpool.tile([P, fout], F32, name="hs")
                    nc.scalar.activation(
                        hs[:wt, :],
                        hp[:wt, :],
                        AF.Copy,
                        scale=dinv_t[:wt, t : t + 1],
                    )
                    nc.sync.dma_start(
                        out=dst_dram[t * P : t * P + wt, :], in_=hs[:wt, :]
                    )

            def scatter_layer(h_full, fout, bias_t, is_last):
                """For every dst block: gather + one-hot matmul scatter-add."""
                for gi, blocks in enumerate(plan.groups):
                    gt = {}
                    for h in (0, 1):
                        nid = plan.gather_nid[(gi, h)]
                        if nid == 0:
                            continue
                        g_tile = gpool.tile(
                            [P, nid // P, fout], F32, tag="gath", name=f"g{gi}_{h}"
                        )
                        i0 = plan.seg_idx16[(gi, h)]
                        nc.gpsimd.dma_gather(
                            g_tile[:],
                            h_full[h * hb : h * hb + rows[h], :],
                            idx_t[:, i0 : i0 + nid // 16],
                            nid,
                            nid,
                            fout,
                            single_packet=False,
                        )
                        gt[h] = g_tile
                    for b in blocks:
                        wb = min(P, npc - b * P)
                        pb = pscat.tile([P, fout], F32, name="pb")
                        nc.tensor.matmul(
                            pb[:],
                            sqrow_t[0:1, b * P : (b + 1) * P],
                            bias_t[:],
                            start=True,
                            stop=False,
                        )
                        nchunks = sum(
                            int(plan.SZ[b, h]) // P for h in (0, 1)
                        )
                        k = 0
                        for h in (0, 1):
                            nch = int(plan.SZ[b, h]) // P
                            if nch == 0:
                                continue
                            col0 = plan.seg_col.get((b, h), 0)
                            ci0 = plan.seg_ci.get((b, h), 0)
                            sel = spool.tile(
                                [P, nch, P], F32, tag="sel", name="sel"
                            )
                            nc.vector.tensor_tensor(
                                out=sel[:],
                                in0=slots_t[:, col0 : col0 + nch].to_broadcast(
                                    [P, nch, P]
                                ),
                                in1=iota_t[:, :]
                                .rearrange("p (a b) -> p a b", a=1)
                                .to_broadcast([P, nch, P]),
                                op=mybir.AluOpType.is_equal,
                            )
                            for ci in range(nch):
                                k += 1
                                nc.tensor.matmul(
                                    pb[:],
                                    sel[:, ci, :],
                                    gt[h][:, ci0 + ci, :],
                                    start=False,
                                    stop=(k == nchunks),
                                )
                        ob = stpool.tile([P, fout], F32, tag="ob", name="ob")
                        if is_last:
                            nc.scalar.activation(
                                ob[:wb, :],
                                pb[:wb, :],
                                AF.Copy,
                                scale=dinv_t[:wb, b : b + 1],
                            )
                            nc.sync.dma_start(
                                out=y_d[b * P : b * P + wb, :], in_=ob[:wb, :]
                            )
                        else:
                            nc.scalar.activation(
                                ob[:],
                                pb[:],
                                AF.Sigmoid,
                                scale=dinv_t[:, b : b + 1],
                            )
                            pt = ptrans.tile([P, P], F32, name="pt")
                            nc.tensor.transpose(pt[:], ob[:], ident_t[:])
                            nc.vector.tensor_copy(
                                x1T_t[:, b * P : (b + 1) * P], pt[:]
                            )

            # ---- layer 0 ----
            gemm_layer(xT_t, w0_t, f1, h1_loc)
            nc.gpsimd.collective_compute(
                "AllGather",
                mybir.AluOpType.bypass,
                replica_groups=rg,
                ins=[h1_loc[:, :].opt()],
                outs=[h1_full[:, :].opt()],
            )
            scatter_layer(h1_full, f1, b0_t, is_last=False)

            # ---- layer 1 ----
            gemm_layer(x1T_t, w1_t, f2, h2_loc)
            nc.gpsimd.collective_compute(
                "AllGather",
                mybir.AluOpType.bypass,
                replica_groups=rg,
                ins=[h2_loc[:, :].opt()],
                outs=[h2_full[:, :].opt()],
            )
            scatter_layer(h2_full, f2, b1_t, is_last=True)

    nc.compile()
    return nc


def _make_in_maps(x, W0, b0, W1, b1, plan, per_core):
    npc = plan.npc
    x = np.asarray(x, dtype=np.float32)
    shared = dict(
        W0=np.asarray(W0, np.float32).reshape(W0.shape[0], -1),
        W1=np.asarray(W1, np.float32).reshape(W1.shape[0], -1),
        b0=np.asarray(b0, np.float32).reshape(1, -1),
        b1=np.asarray(b1, np.float32).reshape(1, -1),
        iota=np.tile(np.arange(P, dtype=np.float32)[None, :], (P, 1)).copy(),
        ident=np.eye(P, dtype=np.float32),
    )
    in_maps = []
    for c in range(plan.n_cores):
        m = dict(shared)
        m["xT"] = np.ascontiguousarray(x[c * npc : (c + 1) * npc, :].T)
        m["idx16"] = per_core[c]["idx16"]
        m["slots"] = per_core[c]["slots"]
        m["degt"] = per_core[c]["degt"]
        m["degrow"] = per_core[c]["degrow"]
        in_maps.append(m)
    return in_maps


_CACHE = {}


def build(x, edges, W0, b0, W1, b1, n_nodes=N_NODES, n_cores=N_CORES,
          gb=GROUP_BLOCKS):
    """Returns (nc, in_maps, plan). Cached on the edge structure size."""
    plan, per_core = _build_metadata(edges, n_nodes, n_cores, gb)
    key = (n_nodes, n_cores, gb, tuple(plan.SZ.reshape(-1).tolist()))
    if key not in _CACHE:
        _CACHE[key] = _build_nc(plan, x.shape[1], W0.shape[1], W1.shape[1])
    nc = _CACHE[key]
    in_maps = _make_in_maps(x, W0, b0, W1, b1, plan, per_core)
    return nc, in_maps, plan


def kernel(x, edges, W0, b0, W1, b1):
    x = np.asarray(x)
    nc, in_maps, plan = build(x, edges, W0, b0, W1, b1)
    res = run_bass_kernel_spmd(nc, in_maps, list(range(plan.n_cores)))
    y = np.concatenate([r["y"] for r in res.results], axis=0)
    return y.astype(np.float32)
